# revision 18
# baseline (speedup 1.0000x reference)
"""Trainium2 Bass kernel for ChannelAttention1D.

Inputs (full): x (8, 256, 16384) f32, gamma (1,) f32.
  energy = einsum('bit,bjt->bij', x, x)
  att    = softmax(max_j(energy) - energy, axis=-1)
  out    = gamma * einsum('bij,bjt->bit', att, x) + x

Sharding: data-parallel over B across 8 NeuronCores (one batch per core).

Per-core design (C=256, T=16384). All PE matmuls run in fp8e4 DoubleRow
perf mode; output precision is preserved by a residual split
x = x8 + r8 (r8 = fp8(x - fp8(x)), so x8 + r8 carries ~0.4% error,
bf16-class). HBM traffic: 12.6 MiB fp8 in + 8.4 MiB bf16 out, spread
across THREE DMA rings (sync + scalar HWDGE, gpsimd SWDGE) — a single
ring sustains only ~190 GB/s on this part, so ring count, not aggregate
HBM bandwidth, is the transfer wall.

  inputs:  xtp8 = packed transposed fp8 x (energy operand; host-packed,
           no on-device transposes), x8p/r8p = fp8 x and residual in
           DoubleRow kc-interleaved layout (phase-2 moving operands).
  energy:  DR matmuls on kt-window pairs; pe0 = rows 0:128 x all cols,
           pe1 = G11 only; G10 = G01^T via one f32 PE transpose.
  softmax: att row i = exp(rowmin_i - E_i)/rowsum_i == softmax(max-E);
           gamma/rowsum folded into the fp8 att operand, which is
           stored m-grouped so phase-2 stationaries are contiguous
           (fast LDWEIGHTS path).
  phase 2: po = att'T.T @ x8 (+ I.T @ r8 | +r8 fused into the DVE
           copy), att' = I + g*att so the matmul emits x + gamma*att@x
           directly. f32->bf16 copies spread over vector/scalar; the
           +r8 work is split between a second DR matmul (PE) and fused
           DVE adds to balance engine load. bf16 stores on all 3 rings.
           Host upcasts to f32.

With gamma == 0 (the shipped input distribution) the kernel output is
(x8 + r8) rounded to bf16, rel err ~4.5e-3 vs the f32 reference; the
attention path itself is exercised via GAMMA1=1 in test.py.
"""

import os

import numpy as np
import ml_dtypes

import concourse.bacc as bacc
import concourse.bass as bass
import concourse.mybir as mybir
import concourse.tile as tile
from concourse.bass_utils import run_bass_kernel_spmd

F32 = mybir.dt.float32
BF16 = mybir.dt.bfloat16
FP8 = mybir.dt.float8e4
NP_FP8 = ml_dtypes.float8_e4m3

B = 8
C = 256
T = 16384
N_CORES = 8
NKT = T // 128       # 128 kt windows
CH = 2048            # phase-2 chunk width (per kc block)
NCH = T // CH        # 8 chunks
# xtp8 chunk column counts (first two small for an early energy start)
XT_CHUNKS = [2048, 2048] + [4096] * 7
assert sum(XT_CHUNKS) == 2 * T
DR = mybir.MatmulPerfMode.DoubleRow

LAST_RESULTS = None  # BassKernelResults of the most recent run (for test.py)


def _build_nc():
    nc = bacc.Bacc(
        "TRN2",
        target_bir_lowering=False,
        debug=False,
        enable_asserts=False,
        num_devices=N_CORES,
    )
    xt_d = nc.dram_tensor("xtp8", [128, 2 * T], FP8, kind="ExternalInput")
    x8_d = nc.dram_tensor("x8p", [128, 2 * T], FP8, kind="ExternalInput")
    r8_d = nc.dram_tensor("r8p", [128, 2 * T], FP8, kind="ExternalInput")
    idb_d = nc.dram_tensor("idb", [128, 128], BF16, kind="ExternalInput")
    id32_d = nc.dram_tensor("id32", [128, 128], F32, kind="ExternalInput")
    diag_d = nc.dram_tensor("diag8", [128, 2 * C], FP8, kind="ExternalInput")
    idr_d = nc.dram_tensor("identr", [128, 2 * C], FP8, kind="ExternalInput")
    g_d = nc.dram_tensor("gamma_b", [128, 1], F32, kind="ExternalInput")
    o_d = nc.dram_tensor("out", [C, T], BF16, kind="ExternalOutput")

    Exp = mybir.ActivationFunctionType.Exp
    Copy = mybir.ActivationFunctionType.Copy
    Alu = mybir.AluOpType
    X = mybir.AxisListType.X

    rings = [nc.sync, nc.scalar, nc.gpsimd]

    with tile.TileContext(nc) as tc:
        with (
            tc.tile_pool(name="xsb", bufs=1) as xpool,
            tc.tile_pool(name="xt", bufs=4) as xtpool,
            tc.tile_pool(name="sm", bufs=1) as smpool,
            tc.tile_pool(name="outp", bufs=6) as outpool,
        ):
            idb = smpool.tile([128, 128], BF16, tag="idb", name="idb")
            id32 = smpool.tile([128, 128], F32, tag="id32", name="id32")
            diag = smpool.tile([128, 2 * C], FP8, tag="diag", name="diag")
            idr = smpool.tile([128, 2 * C], FP8, tag="idr", name="idr")
            g128 = smpool.tile([128, 1], F32, tag="g128", name="g128")

            # phase-2 moving operands (DR kc-interleaved chunks), resident
            x8sb = [
                xpool.tile([128, 2 * CH], FP8, tag=f"x8_{c}", name=f"x8_{c}")
                for c in range(NCH)
            ]
            r8sb = [
                xpool.tile([128, 2 * CH], FP8, tag=f"r8_{c}", name=f"r8_{c}")
                for c in range(NCH)
            ]

            with (
                tc.tile_pool(name="pt", bufs=1, space=bass.MemorySpace.PSUM) as ptpool,
                tc.tile_pool(name="pe", bufs=1, space=bass.MemorySpace.PSUM) as pepool,
            ):
                pe0 = pepool.tile([128, C], F32, tag="pe0", name="pe0")
                pe1 = pepool.tile([128, 128], F32, tag="pe1", name="pe1")

                # ---- phase 1: xtp8 chunk loads (3 rings) + DR energy ----
                xt_tiles = []
                col = 0
                pr = 0
                npairs = 2 * T // 512
                for ci, cols in enumerate(XT_CHUNKS):
                    xt = xtpool.tile([128, cols], FP8, tag="xt", name="xt")
                    rings[ci % 3].dma_start(
                        xt[:], xt_d.ap()[:, col:col + cols]
                    )
                    xt_tiles.append(xt)
                    col += cols
                    v = xt[:].rearrange("p (k c2) -> p k c2", c2=C)
                    for a in range(cols // 512):
                        sl = v[:, 2 * a:2 * a + 2, :]
                        nc.tensor.matmul(
                            pe0[:], sl[:, :, 0:128], sl,
                            perf_mode=DR,
                            start=(pr == 0), stop=(pr == npairs - 1),
                        )
                        nc.tensor.matmul(
                            pe1[:], sl[:, :, 128:256], sl[:, :, 128:256],
                            perf_mode=DR,
                            start=(pr == 0), stop=(pr == npairs - 1),
                        )
                        pr += 1

                # constants: only needed from softmax on — emitted AFTER the
                # xtp8 stream so they don't delay phase-1 input
                nc.scalar.dma_start(idb[:], idb_d.ap())
                nc.scalar.dma_start(id32[:], id32_d.ap())
                nc.scalar.dma_start(diag[:], diag_d.ap())
                nc.scalar.dma_start(idr[:], idr_d.ap())
                nc.scalar.dma_start(g128[:], g_d.ap())
                warm = smpool.tile([128, 1], F32, tag="warm", name="warm")
                nc.scalar.activation(warm[:], g128[:], Exp)

                # phase-2 operand loads: each ring's first phase-2 transfer is
                # gated on a late xtp8 chunk of ANOTHER ring so the whole
                # phase-1 stream keeps priority (per-ring transfers are FIFO).
                nc.vector.tensor_copy(x8sb[0][:, 0:1], xt_tiles[-1][:, 0:1])
                nc.vector.tensor_copy(r8sb[0][:, 0:1], xt_tiles[-2][:, 0:1])
                nc.vector.tensor_copy(x8sb[1][:, 0:1], xt_tiles[-3][:, 0:1])
                p2loads = []
                for c in range(NCH):
                    p2loads.append((x8sb[c], x8_d, c))
                    p2loads.append((r8sb[c], r8_d, c))
                for i, (tile_, dram, c) in enumerate(p2loads):
                    rings[i % 3].dma_start(
                        tile_[:], dram.ap()[:, c * 2 * CH:(c + 1) * 2 * CH]
                    )

                # ---- softmax epilogue ----
                # G10 = transpose(pe0[:, 128:256]) via one f32 PE transpose
                sb01 = smpool.tile([128, 128], F32, tag="sb01", name="sb01")
                nc.vector.tensor_copy(sb01[:], pe0[:, 128:256])
                g10t = ptpool.tile([128, 128], F32, tag="g10t", name="g10t")
                nc.tensor.transpose(g10t[:], sb01[:], id32[:])

                rmin0 = smpool.tile([128, 1], F32, tag="rm0", name="rm0")
                nc.vector.tensor_reduce(rmin0[:], pe0[:], axis=X, op=Alu.min)
                e0 = smpool.tile([128, C], F32, tag="e0", name="e0")
                rs0 = smpool.tile([128, 1], F32, tag="rs0", name="rs0")
                nc.scalar.activation(
                    e0[:], pe0[:], Exp, bias=rmin0[:], scale=-1.0,
                    accum_out=rs0[:],
                )

                rm1a = smpool.tile([128, 1], F32, tag="rm1a", name="rm1a")
                rm1b = smpool.tile([128, 1], F32, tag="rm1b", name="rm1b")
                nc.vector.tensor_reduce(rm1a[:], g10t[:], axis=X, op=Alu.min)
                nc.vector.tensor_reduce(rm1b[:], pe1[:], axis=X, op=Alu.min)
                rmin1 = smpool.tile([128, 1], F32, tag="rm1", name="rm1")
                nc.vector.scalar_tensor_tensor(
                    rmin1[:], rm1a[:], 0.0, rm1b[:], op0=Alu.bypass, op1=Alu.min
                )
                e1 = smpool.tile([128, C], F32, tag="e1", name="e1")
                rs1a = smpool.tile([128, 1], F32, tag="rs1a", name="rs1a")
                rs1b = smpool.tile([128, 1], F32, tag="rs1b", name="rs1b")
                nc.scalar.activation(
                    e1[:, 0:128], g10t[:], Exp,
                    bias=rmin1[:], scale=-1.0, accum_out=rs1a[:],
                )
                nc.scalar.activation(
                    e1[:, 128:256], pe1[:], Exp,
                    bias=rmin1[:], scale=-1.0, accum_out=rs1b[:],
                )
                rs1 = smpool.tile([128, 1], F32, tag="rs1", name="rs1")
                nc.vector.scalar_tensor_tensor(
                    rs1[:], rs1a[:], 0.0, rs1b[:], op0=Alu.bypass, op1=Alu.add
                )

                # g_m = gamma / rowsum, folded into the fp8 att operand
                e_bf = []
                for m, (e, rs) in enumerate([(e0, rs0), (e1, rs1)]):
                    ri = smpool.tile([128, 1], F32, tag=f"ri{m}", name=f"ri{m}")
                    nc.vector.reciprocal(ri[:], rs[:])
                    g = smpool.tile([128, 1], F32, tag=f"g{m}", name=f"g{m}")
                    nc.vector.scalar_tensor_tensor(
                        g[:], ri[:], 0.0, g128[:], op0=Alu.bypass, op1=Alu.mult
                    )
                    eb = smpool.tile([128, C], BF16, tag=f"eb{m}", name=f"eb{m}")
                    nc.scalar.activation(eb[:], e[:], Copy, scale=g[:])
                    e_bf.append(eb)

                # att'T[j, m*C + kc*128 + i'] = att_scaled[m*128+i', kc*128+j]
                #                               + (m*128+i' == kc*128+j)
                # m-grouped so each phase-2 stationary slice is CONTIGUOUS
                # (fast LDWEIGHTS path)
                attT = smpool.tile([128, 2 * C], FP8, tag="attT", name="attT")
                for kc in range(2):
                    pt = ptpool.tile([128, C], BF16, tag=f"pt{kc}", name=f"pt{kc}")
                    for mi in range(2):
                        nc.tensor.transpose(
                            pt[:, mi * 128:(mi + 1) * 128],
                            e_bf[mi][:, kc * 128:(kc + 1) * 128],
                            idb[:],
                        )
                    for mi in range(2):
                        nc.vector.scalar_tensor_tensor(
                            attT[:, mi * C + kc * 128:mi * C + (kc + 1) * 128],
                            pt[:, mi * 128:(mi + 1) * 128], 0.0,
                            diag[:, mi * C + kc * 128:mi * C + (kc + 1) * 128],
                            op0=Alu.bypass, op1=Alu.add,
                        )

            # ---- phase 2: po = att'T.T @ x8 (+ r8), m-outer ----
            with tc.tile_pool(
                name="po", bufs=6, space=bass.MemorySpace.PSUM
            ) as popool:
                for m in range(2):
                    av = attT[:, m * C:(m + 1) * C].rearrange(
                        "p (k i) -> p k i", i=128
                    )
                    iv = idr[:, m * C:(m + 1) * C].rearrange(
                        "p (k i) -> p k i", i=128
                    )
                    for w in range(T // 512):
                        c, q = divmod(w, CH // 512)
                        xv = x8sb[c][:].rearrange("p (k t) -> p k t", t=CH)[
                            :, :, q * 512:(q + 1) * 512
                        ]
                        po = popool.tile([128, 512], F32, tag="po", name="po")
                        ob = outpool.tile([128, 512], BF16, tag="ob", name="ob")
                        if w % 3 == 2:
                            # +r8 fused into the DVE psum->sbuf copy
                            nc.tensor.matmul(
                                po[:], av, xv, perf_mode=DR,
                                start=True, stop=True,
                            )
                            nc.vector.scalar_tensor_tensor(
                                ob[:], po[:], 0.0,
                                r8sb[c][:, m * CH + q * 512:
                                        m * CH + (q + 1) * 512],
                                op0=Alu.bypass, op1=Alu.add,
                            )
                        else:
                            rv = r8sb[c][:].rearrange(
                                "p (k t) -> p k t", t=CH
                            )[:, :, q * 512:(q + 1) * 512]
                            nc.tensor.matmul(
                                po[:], av, xv, perf_mode=DR,
                                start=True, stop=False,
                            )
                            nc.tensor.matmul(
                                po[:], iv, rv, perf_mode=DR,
                                start=False, stop=True,
                            )
                            if w % 3 == 0:
                                nc.vector.tensor_copy(ob[:], po[:])
                            else:
                                nc.scalar.activation(ob[:], po[:], Copy)
                        rings[w % 3].dma_start(
                            o_d.ap()[
                                m * 128:(m + 1) * 128,
                                w * 512:(w + 1) * 512,
                            ],
                            ob[:],
                        )

    nc.compile()
    return nc


_NC_CACHE = None


def _get_nc():
    global _NC_CACHE
    if _NC_CACHE is None:
        _NC_CACHE = _build_nc()
    return _NC_CACHE


def _host_inputs(x, g):
    """Per-batch packed fp8 inputs for one core (x: [C, T] f32)."""
    x8 = x.astype(NP_FP8)
    r = x - x8.astype(np.float32)
    r8 = r.astype(NP_FP8)
    # xtp8[p, kt*256 + c] = x8[c, kt*128 + p]
    xtp = np.ascontiguousarray(
        x8.reshape(C, NKT, 128).transpose(2, 1, 0).reshape(128, 2 * T)
    )
    # x8p[p, c*2CH + kc*CH + t] = x8[kc*128 + p, c*CH + t]
    def pack(a):
        return np.ascontiguousarray(
            a.reshape(2, 128, NCH, CH).transpose(1, 2, 0, 3).reshape(128, 2 * T)
        )
    return xtp, pack(x8), pack(r8)


def kernel(x, gamma):
    x = np.asarray(x, dtype=np.float32)
    g = np.asarray(gamma, dtype=np.float32).reshape(-1)
    assert x.shape == (B, C, T), x.shape

    nc = _get_nc()
    idb = np.eye(128, dtype=ml_dtypes.bfloat16)
    diag = np.zeros((128, 2 * C), dtype=NP_FP8)
    idr = np.zeros((128, 2 * C), dtype=NP_FP8)
    # m-grouped: diag[j, m*C + kc*128 + j] = (m == kc)
    for m in range(2):
        for j in range(128):
            diag[j, m * C + m * 128 + j] = 1.0
    for m in range(2):
        for j in range(128):
            idr[j, m * C + m * 128 + j] = 1.0
    gb = np.full((128, 1), g[0], dtype=np.float32)
    in_maps = []
    for b in range(B):
        xtp, x8p, r8p = _host_inputs(x[b], g)
        in_maps.append(
            {
                "xtp8": xtp,
                "x8p": x8p,
                "r8p": r8p,
                "idb": idb,
                "id32": np.eye(128, dtype=np.float32),
                "diag8": diag,
                "identr": idr,
                "gamma_b": gb,
            }
        )

    trace = os.environ.get("KERNEL_TRACE", "0") == "1"
    res = run_bass_kernel_spmd(
        nc, in_maps, core_ids=list(range(N_CORES)), trace=trace
    )
    global LAST_RESULTS
    LAST_RESULTS = res
    return np.stack(
        [np.asarray(r["out"], dtype=np.float32) for r in res.results], axis=0
    )


# revision 19
# speedup vs baseline: 1.0231x; 1.0231x over previous
"""Trainium2 Bass kernel for ChannelAttention1D.

Inputs (full): x (8, 256, 16384) f32, gamma (1,) f32.
  energy = einsum('bit,bjt->bij', x, x)
  att    = softmax(max_j(energy) - energy, axis=-1)
  out    = gamma * einsum('bij,bjt->bit', att, x) + x

Sharding: data-parallel over B across 8 NeuronCores (one batch per core).

Per-core design (C=256, T=16384). All PE matmuls run in fp8e4 DoubleRow
perf mode; output precision is preserved by a residual split
x = x8 + r8 (r8 = fp8(x - fp8(x)), so x8 + r8 carries ~0.4% error,
bf16-class). HBM traffic: 12.6 MiB fp8 in + 8.4 MiB bf16 out, spread
across THREE DMA rings (sync + scalar HWDGE, gpsimd SWDGE) — a single
ring sustains only ~190 GB/s on this part, so ring count, not aggregate
HBM bandwidth, is the transfer wall.

  inputs:  xtp8 = packed transposed fp8 x (energy operand; host-packed,
           no on-device transposes), x8p/r8p = fp8 x and residual in
           DoubleRow kc-interleaved layout (phase-2 moving operands).
  energy:  DR matmuls on kt-window pairs; pe0 = rows 0:128 x all cols,
           pe1 = G11 only; G10 = G01^T via one f32 PE transpose.
  softmax: att row i = exp(rowmin_i - E_i)/rowsum_i == softmax(max-E);
           gamma/rowsum folded into the fp8 att operand, which is
           stored m-grouped so phase-2 stationaries are contiguous
           (fast LDWEIGHTS path).
  phase 2: po = att'T.T @ x8 (+ I.T @ r8 | +r8 fused into the DVE
           copy), att' = I + g*att so the matmul emits x + gamma*att@x
           directly. f32->bf16 copies spread over vector/scalar; the
           +r8 work is split between a second DR matmul (PE) and fused
           DVE adds to balance engine load. bf16 stores on all 3 rings.
           Host upcasts to f32.

With gamma == 0 (the shipped input distribution) the kernel output is
(x8 + r8) rounded to bf16, rel err ~4.5e-3 vs the f32 reference; the
attention path itself is exercised via GAMMA1=1 in test.py.
"""

import os

import numpy as np
import ml_dtypes

import concourse.bacc as bacc
import concourse.bass as bass
import concourse.mybir as mybir
import concourse.tile as tile
from concourse.bass_utils import run_bass_kernel_spmd

F32 = mybir.dt.float32
BF16 = mybir.dt.bfloat16
FP8 = mybir.dt.float8e4
NP_FP8 = ml_dtypes.float8_e4m3

B = 8
C = 256
T = 16384
N_CORES = 8
NKT = T // 128       # 128 kt windows
CH = 2048            # phase-2 chunk width (per kc block)
NCH = T // CH        # 8 chunks
XTCH = 4096          # xtp8 chunk cols
NXT = 2 * T // XTCH  # 8 chunks
DR = mybir.MatmulPerfMode.DoubleRow

LAST_RESULTS = None  # BassKernelResults of the most recent run (for test.py)


def _build_nc():
    nc = bacc.Bacc(
        "TRN2",
        target_bir_lowering=False,
        debug=False,
        enable_asserts=False,
        num_devices=N_CORES,
    )
    xt_d = nc.dram_tensor("xtp8", [128, 2 * T], FP8, kind="ExternalInput")
    x8_d = nc.dram_tensor("x8p", [128, 2 * T], FP8, kind="ExternalInput")
    r8_d = nc.dram_tensor("r8p", [128, 2 * T], FP8, kind="ExternalInput")
    idb_d = nc.dram_tensor("idb", [128, 128], BF16, kind="ExternalInput")
    id32_d = nc.dram_tensor("id32", [128, 128], F32, kind="ExternalInput")
    diag_d = nc.dram_tensor("diag8", [128, 2 * C], FP8, kind="ExternalInput")
    idr_d = nc.dram_tensor("identr", [128, 2 * C], FP8, kind="ExternalInput")
    g_d = nc.dram_tensor("gamma_b", [128, 1], F32, kind="ExternalInput")
    o_d = nc.dram_tensor("out", [C, T], BF16, kind="ExternalOutput")

    Exp = mybir.ActivationFunctionType.Exp
    Copy = mybir.ActivationFunctionType.Copy
    Alu = mybir.AluOpType
    X = mybir.AxisListType.X

    rings = [nc.sync, nc.scalar]
    srings = [nc.sync, nc.scalar, nc.gpsimd]

    with tile.TileContext(nc) as tc:
        with (
            tc.tile_pool(name="xsb", bufs=1) as xpool,
            tc.tile_pool(name="xt", bufs=3) as xtpool,
            tc.tile_pool(name="sm", bufs=1) as smpool,
            tc.tile_pool(name="outp", bufs=4) as outpool,
        ):
            idb = smpool.tile([128, 128], BF16, tag="idb", name="idb")
            id32 = smpool.tile([128, 128], F32, tag="id32", name="id32")
            diag = smpool.tile([128, 2 * C], FP8, tag="diag", name="diag")
            idr = smpool.tile([128, 2 * C], FP8, tag="idr", name="idr")
            g128 = smpool.tile([128, 1], F32, tag="g128", name="g128")

            # phase-2 moving operands (DR kc-interleaved chunks), resident
            x8sb = [
                xpool.tile([128, 2 * CH], FP8, tag=f"x8_{c}", name=f"x8_{c}")
                for c in range(NCH)
            ]
            r8sb = [
                xpool.tile([128, 2 * CH], FP8, tag=f"r8_{c}", name=f"r8_{c}")
                for c in range(NCH)
            ]

            with (
                tc.tile_pool(name="pt", bufs=1, space=bass.MemorySpace.PSUM) as ptpool,
                tc.tile_pool(name="pe", bufs=1, space=bass.MemorySpace.PSUM) as pepool,
            ):
                pe0 = pepool.tile([128, C], F32, tag="pe0", name="pe0")
                pe1 = pepool.tile([128, 128], F32, tag="pe1", name="pe1")

                # ---- phase 1: xtp8 chunk loads (3 rings) + DR energy ----
                xt_tiles = []
                pr = 0
                npairs = 2 * T // 512
                for ci in range(NXT):
                    xt = xtpool.tile([128, XTCH], FP8, tag="xt", name="xt")
                    rings[ci % 2].dma_start(
                        xt[:], xt_d.ap()[:, ci * XTCH:(ci + 1) * XTCH]
                    )
                    xt_tiles.append(xt)
                    v = xt[:].rearrange("p (k c2) -> p k c2", c2=C)
                    for a in range(XTCH // 512):
                        sl = v[:, 2 * a:2 * a + 2, :]
                        nc.tensor.matmul(
                            pe0[:], sl[:, :, 0:128], sl,
                            perf_mode=DR,
                            start=(pr == 0), stop=(pr == npairs - 1),
                        )
                        nc.tensor.matmul(
                            pe1[:], sl[:, :, 128:256], sl[:, :, 128:256],
                            perf_mode=DR,
                            start=(pr == 0), stop=(pr == npairs - 1),
                        )
                        pr += 1

                # constants: only needed from softmax on — emitted AFTER the
                # xtp8 stream so they don't delay phase-1 input
                nc.scalar.dma_start(idb[:], idb_d.ap())
                nc.scalar.dma_start(id32[:], id32_d.ap())
                nc.scalar.dma_start(diag[:], diag_d.ap())
                nc.scalar.dma_start(idr[:], idr_d.ap())
                nc.scalar.dma_start(g128[:], g_d.ap())
                warm = smpool.tile([128, 1], F32, tag="warm", name="warm")
                nc.scalar.activation(warm[:], g128[:], Exp)

                # phase-2 operand loads: each ring's first phase-2 transfer is
                # gated on a late xtp8 chunk of ANOTHER ring so the whole
                # phase-1 stream keeps priority (per-ring transfers are FIFO).
                nc.vector.tensor_copy(x8sb[0][:, 0:1], xt_tiles[-1][:, 0:1])
                nc.vector.tensor_copy(r8sb[0][:, 0:1], xt_tiles[-2][:, 0:1])
                for c in range(NCH):
                    rings[c % 2].dma_start(
                        x8sb[c][:], x8_d.ap()[:, c * 2 * CH:(c + 1) * 2 * CH]
                    )
                    rings[(c + 1) % 2].dma_start(
                        r8sb[c][:], r8_d.ap()[:, c * 2 * CH:(c + 1) * 2 * CH]
                    )

                # ---- softmax epilogue ----
                # G10 = transpose(pe0[:, 128:256]) via one f32 PE transpose
                sb01 = smpool.tile([128, 128], F32, tag="sb01", name="sb01")
                nc.vector.tensor_copy(sb01[:], pe0[:, 128:256])
                g10t = ptpool.tile([128, 128], F32, tag="g10t", name="g10t")
                nc.tensor.transpose(g10t[:], sb01[:], id32[:])

                rmin0 = smpool.tile([128, 1], F32, tag="rm0", name="rm0")
                nc.vector.tensor_reduce(rmin0[:], pe0[:], axis=X, op=Alu.min)
                e0 = smpool.tile([128, C], F32, tag="e0", name="e0")
                rs0 = smpool.tile([128, 1], F32, tag="rs0", name="rs0")
                nc.scalar.activation(
                    e0[:], pe0[:], Exp, bias=rmin0[:], scale=-1.0,
                    accum_out=rs0[:],
                )

                rm1a = smpool.tile([128, 1], F32, tag="rm1a", name="rm1a")
                rm1b = smpool.tile([128, 1], F32, tag="rm1b", name="rm1b")
                nc.vector.tensor_reduce(rm1a[:], g10t[:], axis=X, op=Alu.min)
                nc.vector.tensor_reduce(rm1b[:], pe1[:], axis=X, op=Alu.min)
                rmin1 = smpool.tile([128, 1], F32, tag="rm1", name="rm1")
                nc.vector.scalar_tensor_tensor(
                    rmin1[:], rm1a[:], 0.0, rm1b[:], op0=Alu.bypass, op1=Alu.min
                )
                e1 = smpool.tile([128, C], F32, tag="e1", name="e1")
                rs1a = smpool.tile([128, 1], F32, tag="rs1a", name="rs1a")
                rs1b = smpool.tile([128, 1], F32, tag="rs1b", name="rs1b")
                nc.scalar.activation(
                    e1[:, 0:128], g10t[:], Exp,
                    bias=rmin1[:], scale=-1.0, accum_out=rs1a[:],
                )
                nc.scalar.activation(
                    e1[:, 128:256], pe1[:], Exp,
                    bias=rmin1[:], scale=-1.0, accum_out=rs1b[:],
                )
                rs1 = smpool.tile([128, 1], F32, tag="rs1", name="rs1")
                nc.vector.scalar_tensor_tensor(
                    rs1[:], rs1a[:], 0.0, rs1b[:], op0=Alu.bypass, op1=Alu.add
                )

                # g_m = gamma / rowsum, folded into the fp8 att operand
                e_bf = []
                for m, (e, rs) in enumerate([(e0, rs0), (e1, rs1)]):
                    ri = smpool.tile([128, 1], F32, tag=f"ri{m}", name=f"ri{m}")
                    nc.vector.reciprocal(ri[:], rs[:])
                    g = smpool.tile([128, 1], F32, tag=f"g{m}", name=f"g{m}")
                    nc.vector.scalar_tensor_tensor(
                        g[:], ri[:], 0.0, g128[:], op0=Alu.bypass, op1=Alu.mult
                    )
                    eb = smpool.tile([128, C], BF16, tag=f"eb{m}", name=f"eb{m}")
                    nc.scalar.activation(eb[:], e[:], Copy, scale=g[:])
                    e_bf.append(eb)

                # att'T[j, m*C + kc*128 + i'] = att_scaled[m*128+i', kc*128+j]
                #                               + (m*128+i' == kc*128+j)
                # m-grouped so each phase-2 stationary slice is CONTIGUOUS
                # (fast LDWEIGHTS path)
                attT = smpool.tile([128, 2 * C], FP8, tag="attT", name="attT")
                for kc in range(2):
                    pt = ptpool.tile([128, C], BF16, tag=f"pt{kc}", name=f"pt{kc}")
                    for mi in range(2):
                        nc.tensor.transpose(
                            pt[:, mi * 128:(mi + 1) * 128],
                            e_bf[mi][:, kc * 128:(kc + 1) * 128],
                            idb[:],
                        )
                    for mi in range(2):
                        nc.vector.scalar_tensor_tensor(
                            attT[:, mi * C + kc * 128:mi * C + (kc + 1) * 128],
                            pt[:, mi * 128:(mi + 1) * 128], 0.0,
                            diag[:, mi * C + kc * 128:mi * C + (kc + 1) * 128],
                            op0=Alu.bypass, op1=Alu.add,
                        )

            # ---- phase 2: po = att'T.T @ x8 (+ r8), m-outer ----
            with tc.tile_pool(
                name="po", bufs=4, space=bass.MemorySpace.PSUM
            ) as popool:
                for m in range(2):
                    av = attT[:, m * C:(m + 1) * C].rearrange(
                        "p (k i) -> p k i", i=128
                    )
                    iv = idr[:, m * C:(m + 1) * C].rearrange(
                        "p (k i) -> p k i", i=128
                    )
                    for w in range(T // 512):
                        c, q = divmod(w, CH // 512)
                        xv = x8sb[c][:].rearrange("p (k t) -> p k t", t=CH)[
                            :, :, q * 512:(q + 1) * 512
                        ]
                        rv = r8sb[c][:].rearrange("p (k t) -> p k t", t=CH)[
                            :, :, q * 512:(q + 1) * 512
                        ]
                        po = popool.tile([128, 512], F32, tag="po", name="po")
                        nc.tensor.matmul(
                            po[:], av, xv, perf_mode=DR, start=True, stop=False
                        )
                        nc.tensor.matmul(
                            po[:], iv, rv, perf_mode=DR, start=False, stop=True
                        )
                        if w % 2 == 0:
                            ob = outpool.tile(
                                [128, 1024], BF16, tag="ob", name="ob"
                            )
                            nc.vector.tensor_copy(ob[:, 0:512], po[:])
                        else:
                            nc.scalar.activation(ob[:, 512:1024], po[:], Copy)
                            srings[(w // 2) % 3].dma_start(
                                o_d.ap()[
                                    m * 128:(m + 1) * 128,
                                    (w - 1) * 512:(w + 1) * 512,
                                ],
                                ob[:],
                            )

    nc.compile()
    return nc


_NC_CACHE = None


def _get_nc():
    global _NC_CACHE
    if _NC_CACHE is None:
        _NC_CACHE = _build_nc()
    return _NC_CACHE


def _host_inputs(x, g):
    """Per-batch packed fp8 inputs for one core (x: [C, T] f32)."""
    x8 = x.astype(NP_FP8)
    r = x - x8.astype(np.float32)
    r8 = r.astype(NP_FP8)
    # xtp8[p, kt*256 + c] = x8[c, kt*128 + p]
    xtp = np.ascontiguousarray(
        x8.reshape(C, NKT, 128).transpose(2, 1, 0).reshape(128, 2 * T)
    )
    # x8p[p, c*2CH + kc*CH + t] = x8[kc*128 + p, c*CH + t]
    def pack(a):
        return np.ascontiguousarray(
            a.reshape(2, 128, NCH, CH).transpose(1, 2, 0, 3).reshape(128, 2 * T)
        )
    return xtp, pack(x8), pack(r8)


def kernel(x, gamma):
    x = np.asarray(x, dtype=np.float32)
    g = np.asarray(gamma, dtype=np.float32).reshape(-1)
    assert x.shape == (B, C, T), x.shape

    nc = _get_nc()
    idb = np.eye(128, dtype=ml_dtypes.bfloat16)
    diag = np.zeros((128, 2 * C), dtype=NP_FP8)
    idr = np.zeros((128, 2 * C), dtype=NP_FP8)
    # m-grouped: diag[j, m*C + kc*128 + j] = (m == kc)
    for m in range(2):
        for j in range(128):
            diag[j, m * C + m * 128 + j] = 1.0
    for m in range(2):
        for j in range(128):
            idr[j, m * C + m * 128 + j] = 1.0
    gb = np.full((128, 1), g[0], dtype=np.float32)
    in_maps = []
    for b in range(B):
        xtp, x8p, r8p = _host_inputs(x[b], g)
        in_maps.append(
            {
                "xtp8": xtp,
                "x8p": x8p,
                "r8p": r8p,
                "idb": idb,
                "id32": np.eye(128, dtype=np.float32),
                "diag8": diag,
                "identr": idr,
                "gamma_b": gb,
            }
        )

    trace = os.environ.get("KERNEL_TRACE", "0") == "1"
    res = run_bass_kernel_spmd(
        nc, in_maps, core_ids=list(range(N_CORES)), trace=trace
    )
    global LAST_RESULTS
    LAST_RESULTS = res
    return np.stack(
        [np.asarray(r["out"], dtype=np.float32) for r in res.results], axis=0
    )


# revision 20
# speedup vs baseline: 1.1354x; 1.1098x over previous
"""Trainium2 Bass kernel for ChannelAttention1D.

Inputs (full): x (8, 256, 16384) f32, gamma (1,) f32.
  energy = einsum('bit,bjt->bij', x, x)
  att    = softmax(max_j(energy) - energy, axis=-1)
  out    = gamma * einsum('bij,bjt->bit', att, x) + x

Sharding: data-parallel over B across 8 NeuronCores (one batch per core).

Per-core design (C=256, T=16384). All PE matmuls run in fp8e4 DoubleRow
perf mode; output precision is preserved by a residual split
x = x8 + r8 (r8 = fp8(x - fp8(x)), so x8 + r8 carries ~0.4% error,
bf16-class). HBM traffic: 12.6 MiB fp8 in + 8.4 MiB bf16 out, spread
across THREE DMA rings (sync + scalar HWDGE, gpsimd SWDGE) — a single
ring sustains only ~190 GB/s on this part, so ring count, not aggregate
HBM bandwidth, is the transfer wall.

  inputs:  xtp8 = packed transposed fp8 x (energy operand; host-packed,
           no on-device transposes), x8p/r8p = fp8 x and residual in
           DoubleRow kc-interleaved layout (phase-2 moving operands).
  energy:  DR matmuls on kt-window pairs; pe0 = rows 0:128 x all cols,
           pe1 = G11 only; G10 = G01^T via one f32 PE transpose.
  softmax: att row i = exp(rowmin_i - E_i)/rowsum_i == softmax(max-E);
           gamma/rowsum folded into the fp8 att operand, which is
           stored m-grouped so phase-2 stationaries are contiguous
           (fast LDWEIGHTS path).
  phase 2: po = att'T.T @ x8 (+ I.T @ r8 | +r8 fused into the DVE
           copy), att' = I + g*att so the matmul emits x + gamma*att@x
           directly. f32->bf16 copies spread over vector/scalar; the
           +r8 work is split between a second DR matmul (PE) and fused
           DVE adds to balance engine load. bf16 stores on all 3 rings.
           Host upcasts to f32.

With gamma == 0 (the shipped input distribution) the kernel output is
(x8 + r8) rounded to bf16, rel err ~4.5e-3 vs the f32 reference; the
attention path itself is exercised via GAMMA1=1 in test.py.
"""

import os

import numpy as np
import ml_dtypes

import concourse.bacc as bacc
import concourse.bass as bass
import concourse.mybir as mybir
import concourse.tile as tile
from concourse.bass_utils import run_bass_kernel_spmd

F32 = mybir.dt.float32
BF16 = mybir.dt.bfloat16
FP8 = mybir.dt.float8e4
NP_FP8 = ml_dtypes.float8_e4m3

B = 8
C = 256
T = 16384
N_CORES = 8
NKT = T // 128       # 128 kt windows
CH = 2048            # phase-2 chunk width (per kc block)
NCH = T // CH        # 8 chunks
XTCH = 4096          # xtp8 chunk cols
NXT = 2 * T // XTCH  # 8 chunks
DR = mybir.MatmulPerfMode.DoubleRow

LAST_RESULTS = None  # BassKernelResults of the most recent run (for test.py)


def _build_nc():
    nc = bacc.Bacc(
        "TRN2",
        target_bir_lowering=False,
        debug=False,
        enable_asserts=False,
        num_devices=N_CORES,
    )
    xt_d = nc.dram_tensor("xtp8", [128, 2 * T], FP8, kind="ExternalInput")
    x8_d = nc.dram_tensor("x8p", [128, 2 * T], FP8, kind="ExternalInput")
    r8_d = nc.dram_tensor("r8p", [128, 2 * T], FP8, kind="ExternalInput")
    idb_d = nc.dram_tensor("idb", [128, 128], BF16, kind="ExternalInput")
    id32_d = nc.dram_tensor("id32", [128, 128], F32, kind="ExternalInput")
    diag_d = nc.dram_tensor("diag8", [128, 2 * C], FP8, kind="ExternalInput")
    idr_d = nc.dram_tensor("identr", [128, 2 * C], FP8, kind="ExternalInput")
    g_d = nc.dram_tensor("gamma_b", [128, 1], F32, kind="ExternalInput")
    o_d = nc.dram_tensor("out", [C, T], BF16, kind="ExternalOutput")

    Exp = mybir.ActivationFunctionType.Exp
    Copy = mybir.ActivationFunctionType.Copy
    Alu = mybir.AluOpType
    X = mybir.AxisListType.X

    rings = [nc.sync, nc.scalar]
    srings = [nc.sync, nc.scalar, nc.gpsimd]

    with tile.TileContext(nc) as tc:
        with (
            tc.tile_pool(name="xsb", bufs=1) as xpool,
            tc.tile_pool(name="xt", bufs=3) as xtpool,
            tc.tile_pool(name="sm", bufs=1) as smpool,
            tc.tile_pool(name="outp", bufs=4) as outpool,
        ):
            idb = smpool.tile([128, 128], BF16, tag="idb", name="idb")
            id32 = smpool.tile([128, 128], F32, tag="id32", name="id32")
            diag = smpool.tile([128, 2 * C], FP8, tag="diag", name="diag")
            idr = smpool.tile([128, 2 * C], FP8, tag="idr", name="idr")
            g128 = smpool.tile([128, 1], F32, tag="g128", name="g128")

            # phase-2 moving operands (DR kc-interleaved chunks), resident
            x8sb = [
                xpool.tile([128, 2 * CH], FP8, tag=f"x8_{c}", name=f"x8_{c}")
                for c in range(NCH)
            ]
            r8sb = [
                xpool.tile([128, 2 * CH], FP8, tag=f"r8_{c}", name=f"r8_{c}")
                for c in range(NCH)
            ]

            with (
                tc.tile_pool(name="pt", bufs=1, space=bass.MemorySpace.PSUM) as ptpool,
                tc.tile_pool(name="pe", bufs=1, space=bass.MemorySpace.PSUM) as pepool,
            ):
                pe0 = pepool.tile([128, C], F32, tag="pe0", name="pe0")
                pe1 = pepool.tile([128, 128], F32, tag="pe1", name="pe1")

                # ---- phase 1: xtp8 chunk loads (3 rings) + DR energy ----
                xt_tiles = []
                pr = 0
                npairs = 2 * T // 512
                for ci in range(NXT):
                    xt = xtpool.tile([128, XTCH], FP8, tag="xt", name="xt")
                    rings[ci % 2].dma_start(
                        xt[:], xt_d.ap()[:, ci * XTCH:(ci + 1) * XTCH]
                    )
                    xt_tiles.append(xt)
                    v = xt[:].rearrange("p (k c2) -> p k c2", c2=C)
                    for a in range(XTCH // 512):
                        sl = v[:, 2 * a:2 * a + 2, :]
                        nc.tensor.matmul(
                            pe0[:], sl[:, :, 0:128], sl,
                            perf_mode=DR,
                            start=(pr == 0), stop=(pr == npairs - 1),
                        )
                        nc.tensor.matmul(
                            pe1[:], sl[:, :, 128:256], sl[:, :, 128:256],
                            perf_mode=DR,
                            start=(pr == 0), stop=(pr == npairs - 1),
                        )
                        pr += 1

                # constants: only needed from softmax on — emitted AFTER the
                # xtp8 stream so they don't delay phase-1 input
                nc.scalar.dma_start(idb[:], idb_d.ap())
                nc.scalar.dma_start(id32[:], id32_d.ap())
                nc.scalar.dma_start(diag[:], diag_d.ap())
                nc.scalar.dma_start(idr[:], idr_d.ap())
                nc.scalar.dma_start(g128[:], g_d.ap())
                warm = smpool.tile([128, 1], F32, tag="warm", name="warm")
                nc.scalar.activation(warm[:], g128[:], Exp)

                # phase-2 operand loads: each ring's first phase-2 transfer is
                # gated on a late xtp8 chunk of ANOTHER ring so the whole
                # phase-1 stream keeps priority (per-ring transfers are FIFO).
                nc.vector.tensor_copy(x8sb[0][:, 0:1], xt_tiles[-1][:, 0:1])
                nc.vector.tensor_copy(r8sb[0][:, 0:1], xt_tiles[-2][:, 0:1])
                for c in range(NCH):
                    rings[c % 2].dma_start(
                        x8sb[c][:], x8_d.ap()[:, c * 2 * CH:(c + 1) * 2 * CH]
                    )
                    rings[(c + 1) % 2].dma_start(
                        r8sb[c][:], r8_d.ap()[:, c * 2 * CH:(c + 1) * 2 * CH]
                    )

                # ---- softmax epilogue ----
                # G10 = transpose(pe0[:, 128:256]) via one f32 PE transpose
                sb01 = smpool.tile([128, 128], F32, tag="sb01", name="sb01")
                nc.vector.tensor_copy(sb01[:], pe0[:, 128:256])
                g10t = ptpool.tile([128, 128], F32, tag="g10t", name="g10t")
                nc.tensor.transpose(g10t[:], sb01[:], id32[:])

                rmin0 = smpool.tile([128, 1], F32, tag="rm0", name="rm0")
                nc.vector.tensor_reduce(rmin0[:], pe0[:], axis=X, op=Alu.min)
                e0 = smpool.tile([128, C], F32, tag="e0", name="e0")
                rs0 = smpool.tile([128, 1], F32, tag="rs0", name="rs0")
                nc.scalar.activation(
                    e0[:], pe0[:], Exp, bias=rmin0[:], scale=-1.0,
                    accum_out=rs0[:],
                )

                rm1a = smpool.tile([128, 1], F32, tag="rm1a", name="rm1a")
                rm1b = smpool.tile([128, 1], F32, tag="rm1b", name="rm1b")
                nc.vector.tensor_reduce(rm1a[:], g10t[:], axis=X, op=Alu.min)
                nc.vector.tensor_reduce(rm1b[:], pe1[:], axis=X, op=Alu.min)
                rmin1 = smpool.tile([128, 1], F32, tag="rm1", name="rm1")
                nc.vector.scalar_tensor_tensor(
                    rmin1[:], rm1a[:], 0.0, rm1b[:], op0=Alu.bypass, op1=Alu.min
                )
                e1 = smpool.tile([128, C], F32, tag="e1", name="e1")
                rs1a = smpool.tile([128, 1], F32, tag="rs1a", name="rs1a")
                rs1b = smpool.tile([128, 1], F32, tag="rs1b", name="rs1b")
                nc.scalar.activation(
                    e1[:, 0:128], g10t[:], Exp,
                    bias=rmin1[:], scale=-1.0, accum_out=rs1a[:],
                )
                nc.scalar.activation(
                    e1[:, 128:256], pe1[:], Exp,
                    bias=rmin1[:], scale=-1.0, accum_out=rs1b[:],
                )
                rs1 = smpool.tile([128, 1], F32, tag="rs1", name="rs1")
                nc.vector.scalar_tensor_tensor(
                    rs1[:], rs1a[:], 0.0, rs1b[:], op0=Alu.bypass, op1=Alu.add
                )

                # g_m = gamma / rowsum, folded into the fp8 att operand
                e_bf = []
                for m, (e, rs) in enumerate([(e0, rs0), (e1, rs1)]):
                    ri = smpool.tile([128, 1], F32, tag=f"ri{m}", name=f"ri{m}")
                    nc.vector.reciprocal(ri[:], rs[:])
                    g = smpool.tile([128, 1], F32, tag=f"g{m}", name=f"g{m}")
                    nc.vector.scalar_tensor_tensor(
                        g[:], ri[:], 0.0, g128[:], op0=Alu.bypass, op1=Alu.mult
                    )
                    eb = smpool.tile([128, C], BF16, tag=f"eb{m}", name=f"eb{m}")
                    nc.scalar.activation(eb[:], e[:], Copy, scale=g[:])
                    e_bf.append(eb)

                # att'T[j, m*C + kc*128 + i'] = att_scaled[m*128+i', kc*128+j]
                #                               + (m*128+i' == kc*128+j)
                # m-grouped so each phase-2 stationary slice is CONTIGUOUS
                # (fast LDWEIGHTS path)
                attT = smpool.tile([128, 2 * C], FP8, tag="attT", name="attT")
                for kc in range(2):
                    pt = ptpool.tile([128, C], BF16, tag=f"pt{kc}", name=f"pt{kc}")
                    for mi in range(2):
                        nc.tensor.transpose(
                            pt[:, mi * 128:(mi + 1) * 128],
                            e_bf[mi][:, kc * 128:(kc + 1) * 128],
                            idb[:],
                        )
                    for mi in range(2):
                        nc.vector.scalar_tensor_tensor(
                            attT[:, mi * C + kc * 128:mi * C + (kc + 1) * 128],
                            pt[:, mi * 128:(mi + 1) * 128], 0.0,
                            diag[:, mi * C + kc * 128:mi * C + (kc + 1) * 128],
                            op0=Alu.bypass, op1=Alu.add,
                        )

            # ---- phase 2: po = att'T.T @ x8 (+ r8), m-outer ----
            with tc.tile_pool(
                name="po", bufs=4, space=bass.MemorySpace.PSUM
            ) as popool:
                for m in range(2):
                    av = attT[:, m * C:(m + 1) * C].rearrange(
                        "p (k i) -> p k i", i=128
                    )
                    iv = idr[:, m * C:(m + 1) * C].rearrange(
                        "p (k i) -> p k i", i=128
                    )
                    for w in range(T // 512):
                        c, q = divmod(w, CH // 512)
                        xv = x8sb[c][:].rearrange("p (k t) -> p k t", t=CH)[
                            :, :, q * 512:(q + 1) * 512
                        ]
                        rv = r8sb[c][:].rearrange("p (k t) -> p k t", t=CH)[
                            :, :, q * 512:(q + 1) * 512
                        ]
                        po = popool.tile([128, 512], F32, tag="po", name="po")
                        nc.tensor.matmul(
                            po[:], av, xv, perf_mode=DR, start=True, stop=False
                        )
                        nc.tensor.matmul(
                            po[:], iv, rv, perf_mode=DR, start=False, stop=True
                        )
                        if w % 2 == 0:
                            ob = outpool.tile(
                                [128, 1024], BF16, tag="ob", name="ob"
                            )
                            nc.vector.tensor_copy(ob[:, 0:512], po[:])
                        else:
                            nc.scalar.activation(ob[:, 512:1024], po[:], Copy)
                            rings[(w // 2) % 2].dma_start(
                                o_d.ap()[
                                    m * 128:(m + 1) * 128,
                                    (w - 1) * 512:(w + 1) * 512,
                                ],
                                ob[:],
                            )

    nc.compile()
    return nc


_NC_CACHE = None


def _get_nc():
    global _NC_CACHE
    if _NC_CACHE is None:
        _NC_CACHE = _build_nc()
    return _NC_CACHE


def _host_inputs(x, g):
    """Per-batch packed fp8 inputs for one core (x: [C, T] f32)."""
    x8 = x.astype(NP_FP8)
    r = x - x8.astype(np.float32)
    r8 = r.astype(NP_FP8)
    # xtp8[p, kt*256 + c] = x8[c, kt*128 + p]
    xtp = np.ascontiguousarray(
        x8.reshape(C, NKT, 128).transpose(2, 1, 0).reshape(128, 2 * T)
    )
    # x8p[p, c*2CH + kc*CH + t] = x8[kc*128 + p, c*CH + t]
    def pack(a):
        return np.ascontiguousarray(
            a.reshape(2, 128, NCH, CH).transpose(1, 2, 0, 3).reshape(128, 2 * T)
        )
    return xtp, pack(x8), pack(r8)


def kernel(x, gamma):
    x = np.asarray(x, dtype=np.float32)
    g = np.asarray(gamma, dtype=np.float32).reshape(-1)
    assert x.shape == (B, C, T), x.shape

    nc = _get_nc()
    idb = np.eye(128, dtype=ml_dtypes.bfloat16)
    diag = np.zeros((128, 2 * C), dtype=NP_FP8)
    idr = np.zeros((128, 2 * C), dtype=NP_FP8)
    # m-grouped: diag[j, m*C + kc*128 + j] = (m == kc)
    for m in range(2):
        for j in range(128):
            diag[j, m * C + m * 128 + j] = 1.0
    for m in range(2):
        for j in range(128):
            idr[j, m * C + m * 128 + j] = 1.0
    gb = np.full((128, 1), g[0], dtype=np.float32)
    in_maps = []
    for b in range(B):
        xtp, x8p, r8p = _host_inputs(x[b], g)
        in_maps.append(
            {
                "xtp8": xtp,
                "x8p": x8p,
                "r8p": r8p,
                "idb": idb,
                "id32": np.eye(128, dtype=np.float32),
                "diag8": diag,
                "identr": idr,
                "gamma_b": gb,
            }
        )

    trace = os.environ.get("KERNEL_TRACE", "0") == "1"
    res = run_bass_kernel_spmd(
        nc, in_maps, core_ids=list(range(N_CORES)), trace=trace
    )
    global LAST_RESULTS
    LAST_RESULTS = res
    return np.stack(
        [np.asarray(r["out"], dtype=np.float32) for r in res.results], axis=0
    )


# revision 21
# speedup vs baseline: 1.1751x; 1.0350x over previous
"""Trainium2 Bass kernel for ChannelAttention1D.

Inputs (full): x (8, 256, 16384) f32, gamma (1,) f32.
  energy = einsum('bit,bjt->bij', x, x)
  att    = softmax(max_j(energy) - energy, axis=-1)
  out    = gamma * einsum('bij,bjt->bit', att, x) + x

Sharding: data-parallel over B across 8 NeuronCores (one batch per core).

Per-core design (C=256, T=16384). All PE matmuls run in fp8e4 DoubleRow
perf mode; output precision is preserved by a residual split
x = x8 + r8 (r8 = fp8(x - fp8(x)), so x8 + r8 carries ~0.4% error,
bf16-class). HBM traffic: 12.6 MiB fp8 in + 8.4 MiB bf16 out, spread
across THREE DMA rings (sync + scalar HWDGE, gpsimd SWDGE) — a single
ring sustains only ~190 GB/s on this part, so ring count, not aggregate
HBM bandwidth, is the transfer wall.

  inputs:  xtp8 = packed transposed fp8 x (energy operand; host-packed,
           no on-device transposes), x8p/r8p = fp8 x and residual in
           DoubleRow kc-interleaved layout (phase-2 moving operands).
  energy:  DR matmuls on kt-window pairs; pe0 = rows 0:128 x all cols,
           pe1 = G11 only; G10 = G01^T via one f32 PE transpose.
  softmax: att row i = exp(rowmin_i - E_i)/rowsum_i == softmax(max-E);
           gamma/rowsum folded into the fp8 att operand, which is
           stored m-grouped so phase-2 stationaries are contiguous
           (fast LDWEIGHTS path).
  phase 2: po = att'T.T @ x8 (+ I.T @ r8 | +r8 fused into the DVE
           copy), att' = I + g*att so the matmul emits x + gamma*att@x
           directly. f32->bf16 copies spread over vector/scalar; the
           +r8 work is split between a second DR matmul (PE) and fused
           DVE adds to balance engine load. bf16 stores on all 3 rings.
           Host upcasts to f32.

With gamma == 0 (the shipped input distribution) the kernel output is
(x8 + r8) rounded to bf16, rel err ~4.5e-3 vs the f32 reference; the
attention path itself is exercised via GAMMA1=1 in test.py.
"""

import os

import numpy as np
import ml_dtypes

import concourse.bacc as bacc
import concourse.bass as bass
import concourse.mybir as mybir
import concourse.tile as tile
from concourse.bass_utils import run_bass_kernel_spmd

F32 = mybir.dt.float32
BF16 = mybir.dt.bfloat16
FP8 = mybir.dt.float8e4
NP_FP8 = ml_dtypes.float8_e4m3

B = 8
C = 256
T = 16384
N_CORES = 8
NKT = T // 128       # 128 kt windows
CH = 2048            # phase-2 chunk width (per kc block)
NCH = T // CH        # 8 chunks
XTCH = 4096          # xtp8 chunk cols
NXT = 2 * T // XTCH  # 8 chunks
DR = mybir.MatmulPerfMode.DoubleRow

LAST_RESULTS = None  # BassKernelResults of the most recent run (for test.py)


def _build_nc():
    nc = bacc.Bacc(
        "TRN2",
        target_bir_lowering=False,
        debug=False,
        enable_asserts=False,
        num_devices=N_CORES,
    )
    xt_d = nc.dram_tensor("xtp8", [128, 2 * T], FP8, kind="ExternalInput")
    xr8_d = nc.dram_tensor("xr8p", [128, 4 * T], FP8, kind="ExternalInput")
    idb_d = nc.dram_tensor("idb", [128, 128], BF16, kind="ExternalInput")
    id32_d = nc.dram_tensor("id32", [128, 128], F32, kind="ExternalInput")
    diag_d = nc.dram_tensor("diag8", [128, 2 * C], FP8, kind="ExternalInput")
    idr_d = nc.dram_tensor("identr", [128, 2 * C], FP8, kind="ExternalInput")
    g_d = nc.dram_tensor("gamma_b", [128, 1], F32, kind="ExternalInput")
    o_d = nc.dram_tensor("out", [C, T], BF16, kind="ExternalOutput")

    Exp = mybir.ActivationFunctionType.Exp
    Copy = mybir.ActivationFunctionType.Copy
    Alu = mybir.AluOpType
    X = mybir.AxisListType.X

    rings = [nc.sync, nc.scalar]
    srings = [nc.sync, nc.scalar, nc.gpsimd]

    with tile.TileContext(nc) as tc:
        with (
            tc.tile_pool(name="xsb", bufs=1) as xpool,
            tc.tile_pool(name="xt", bufs=4) as xtpool,
            tc.tile_pool(name="sm", bufs=1) as smpool,
            tc.tile_pool(name="outp", bufs=6) as outpool,
        ):
            idb = smpool.tile([128, 128], BF16, tag="idb", name="idb")
            id32 = smpool.tile([128, 128], F32, tag="id32", name="id32")
            diag = smpool.tile([128, 2 * C], FP8, tag="diag", name="diag")
            idr = smpool.tile([128, 2 * C], FP8, tag="idr", name="idr")
            g128 = smpool.tile([128, 1], F32, tag="g128", name="g128")

            # phase-2 moving operands: per chunk one merged tile holding
            # [x8 kc0 | x8 kc1 | r8 kc0 | r8 kc1], loaded in one DMA
            xr8sb = [
                xpool.tile([128, 4 * CH], FP8, tag=f"xr8_{c}", name=f"xr8_{c}")
                for c in range(NCH)
            ]

            with (
                tc.tile_pool(name="pt", bufs=1, space=bass.MemorySpace.PSUM) as ptpool,
                tc.tile_pool(name="pe", bufs=1, space=bass.MemorySpace.PSUM) as pepool,
            ):
                pe0 = pepool.tile([128, C], F32, tag="pe0", name="pe0")
                pe1 = pepool.tile([128, 128], F32, tag="pe1", name="pe1")

                # ---- phase 1: xtp8 chunk loads (3 rings) + DR energy ----
                xt_tiles = []
                pr = 0
                npairs = 2 * T // 512
                for ci in range(NXT):
                    xt = xtpool.tile([128, XTCH], FP8, tag="xt", name="xt")
                    rings[ci % 2].dma_start(
                        xt[:], xt_d.ap()[:, ci * XTCH:(ci + 1) * XTCH]
                    )
                    xt_tiles.append(xt)
                    v = xt[:].rearrange("p (k c2) -> p k c2", c2=C)
                    for a in range(XTCH // 512):
                        sl = v[:, 2 * a:2 * a + 2, :]
                        nc.tensor.matmul(
                            pe0[:], sl[:, :, 0:128], sl,
                            perf_mode=DR,
                            start=(pr == 0), stop=(pr == npairs - 1),
                        )
                        nc.tensor.matmul(
                            pe1[:], sl[:, :, 128:256], sl[:, :, 128:256],
                            perf_mode=DR,
                            start=(pr == 0), stop=(pr == npairs - 1),
                        )
                        pr += 1

                # constants: only needed from softmax on — emitted AFTER the
                # xtp8 stream so they don't delay phase-1 input
                nc.scalar.dma_start(idb[:], idb_d.ap())
                nc.scalar.dma_start(id32[:], id32_d.ap())
                nc.scalar.dma_start(diag[:], diag_d.ap())
                nc.scalar.dma_start(idr[:], idr_d.ap())
                nc.scalar.dma_start(g128[:], g_d.ap())
                warm = smpool.tile([128, 1], F32, tag="warm", name="warm")
                nc.scalar.activation(warm[:], g128[:], Exp)

                # phase-2 operand loads: each ring's first phase-2 transfer is
                # gated on a late xtp8 chunk of ANOTHER ring so the whole
                # phase-1 stream keeps priority (per-ring transfers are FIFO).
                nc.vector.tensor_copy(xr8sb[0][:, 0:1], xt_tiles[-1][:, 0:1])
                nc.vector.tensor_copy(xr8sb[1][:, 0:1], xt_tiles[-2][:, 0:1])
                for c in range(NCH):
                    rings[c % 2].dma_start(
                        xr8sb[c][:], xr8_d.ap()[:, c * 4 * CH:(c + 1) * 4 * CH]
                    )

                # ---- softmax epilogue ----
                # G10 = transpose(pe0[:, 128:256]) via one f32 PE transpose
                sb01 = smpool.tile([128, 128], F32, tag="sb01", name="sb01")
                nc.vector.tensor_copy(sb01[:], pe0[:, 128:256])
                g10t = ptpool.tile([128, 128], F32, tag="g10t", name="g10t")
                nc.tensor.transpose(g10t[:], sb01[:], id32[:])

                rmin0 = smpool.tile([128, 1], F32, tag="rm0", name="rm0")
                nc.vector.tensor_reduce(rmin0[:], pe0[:], axis=X, op=Alu.min)
                e0 = smpool.tile([128, C], F32, tag="e0", name="e0")
                rs0 = smpool.tile([128, 1], F32, tag="rs0", name="rs0")
                nc.scalar.activation(
                    e0[:], pe0[:], Exp, bias=rmin0[:], scale=-1.0,
                    accum_out=rs0[:],
                )

                rm1a = smpool.tile([128, 1], F32, tag="rm1a", name="rm1a")
                rm1b = smpool.tile([128, 1], F32, tag="rm1b", name="rm1b")
                nc.vector.tensor_reduce(rm1a[:], g10t[:], axis=X, op=Alu.min)
                nc.vector.tensor_reduce(rm1b[:], pe1[:], axis=X, op=Alu.min)
                rmin1 = smpool.tile([128, 1], F32, tag="rm1", name="rm1")
                nc.vector.scalar_tensor_tensor(
                    rmin1[:], rm1a[:], 0.0, rm1b[:], op0=Alu.bypass, op1=Alu.min
                )
                e1 = smpool.tile([128, C], F32, tag="e1", name="e1")
                rs1a = smpool.tile([128, 1], F32, tag="rs1a", name="rs1a")
                rs1b = smpool.tile([128, 1], F32, tag="rs1b", name="rs1b")
                nc.scalar.activation(
                    e1[:, 0:128], g10t[:], Exp,
                    bias=rmin1[:], scale=-1.0, accum_out=rs1a[:],
                )
                nc.scalar.activation(
                    e1[:, 128:256], pe1[:], Exp,
                    bias=rmin1[:], scale=-1.0, accum_out=rs1b[:],
                )
                rs1 = smpool.tile([128, 1], F32, tag="rs1", name="rs1")
                nc.vector.scalar_tensor_tensor(
                    rs1[:], rs1a[:], 0.0, rs1b[:], op0=Alu.bypass, op1=Alu.add
                )

                # g_m = gamma / rowsum, folded into the fp8 att operand
                e_bf = []
                for m, (e, rs) in enumerate([(e0, rs0), (e1, rs1)]):
                    ri = smpool.tile([128, 1], F32, tag=f"ri{m}", name=f"ri{m}")
                    nc.vector.reciprocal(ri[:], rs[:])
                    g = smpool.tile([128, 1], F32, tag=f"g{m}", name=f"g{m}")
                    nc.vector.scalar_tensor_tensor(
                        g[:], ri[:], 0.0, g128[:], op0=Alu.bypass, op1=Alu.mult
                    )
                    eb = smpool.tile([128, C], BF16, tag=f"eb{m}", name=f"eb{m}")
                    nc.scalar.activation(eb[:], e[:], Copy, scale=g[:])
                    e_bf.append(eb)

                # att'T[j, m*C + kc*128 + i'] = att_scaled[m*128+i', kc*128+j]
                #                               + (m*128+i' == kc*128+j)
                # m-grouped so each phase-2 stationary slice is CONTIGUOUS
                # (fast LDWEIGHTS path)
                attT = smpool.tile([128, 2 * C], FP8, tag="attT", name="attT")
                for kc in range(2):
                    pt = ptpool.tile([128, C], BF16, tag=f"pt{kc}", name=f"pt{kc}")
                    for mi in range(2):
                        nc.tensor.transpose(
                            pt[:, mi * 128:(mi + 1) * 128],
                            e_bf[mi][:, kc * 128:(kc + 1) * 128],
                            idb[:],
                        )
                    for mi in range(2):
                        nc.vector.scalar_tensor_tensor(
                            attT[:, mi * C + kc * 128:mi * C + (kc + 1) * 128],
                            pt[:, mi * 128:(mi + 1) * 128], 0.0,
                            diag[:, mi * C + kc * 128:mi * C + (kc + 1) * 128],
                            op0=Alu.bypass, op1=Alu.add,
                        )

            # ---- phase 2: po = att'T.T @ x8 (+ r8), m-outer ----
            with tc.tile_pool(
                name="po", bufs=6, space=bass.MemorySpace.PSUM
            ) as popool:
                for m in range(2):
                    av = attT[:, m * C:(m + 1) * C].rearrange(
                        "p (k i) -> p k i", i=128
                    )
                    iv = idr[:, m * C:(m + 1) * C].rearrange(
                        "p (k i) -> p k i", i=128
                    )
                    for w in range(T // 512):
                        c, q = divmod(w, CH // 512)
                        xv = xr8sb[c][:, 0:2 * CH].rearrange(
                            "p (k t) -> p k t", t=CH
                        )[:, :, q * 512:(q + 1) * 512]
                        rv = xr8sb[c][:, 2 * CH:4 * CH].rearrange(
                            "p (k t) -> p k t", t=CH
                        )[:, :, q * 512:(q + 1) * 512]
                        po = popool.tile([128, 512], F32, tag="po", name="po")
                        nc.tensor.matmul(
                            po[:], av, xv, perf_mode=DR, start=True, stop=False
                        )
                        nc.tensor.matmul(
                            po[:], iv, rv, perf_mode=DR, start=False, stop=True
                        )
                        if w % 2 == 0:
                            ob = outpool.tile(
                                [128, 1024], BF16, tag="ob", name="ob"
                            )
                            nc.vector.tensor_copy(ob[:, 0:512], po[:])
                        else:
                            nc.scalar.activation(ob[:, 512:1024], po[:], Copy)
                            rings[(w // 2) % 2].dma_start(
                                o_d.ap()[
                                    m * 128:(m + 1) * 128,
                                    (w - 1) * 512:(w + 1) * 512,
                                ],
                                ob[:],
                            )

    nc.compile()
    return nc


_NC_CACHE = None


def _get_nc():
    global _NC_CACHE
    if _NC_CACHE is None:
        _NC_CACHE = _build_nc()
    return _NC_CACHE


def _host_inputs(x, g):
    """Per-batch packed fp8 inputs for one core (x: [C, T] f32)."""
    x8 = x.astype(NP_FP8)
    r = x - x8.astype(np.float32)
    r8 = r.astype(NP_FP8)
    # xtp8[p, kt*256 + c] = x8[c, kt*128 + p]
    xtp = np.ascontiguousarray(
        x8.reshape(C, NKT, 128).transpose(2, 1, 0).reshape(128, 2 * T)
    )
    # merged layout per chunk: [x8 kc0 | x8 kc1 | r8 kc0 | r8 kc1]
    def pack4(a):
        # -> [128, NCH, 2, CH] with [p, c, kc, t] = a[kc*128+p, c*CH+t]
        return a.reshape(2, 128, NCH, CH).transpose(1, 2, 0, 3)
    xr = np.concatenate([pack4(x8), pack4(r8)], axis=2)  # [128, NCH, 4, CH]
    return xtp, np.ascontiguousarray(xr.reshape(128, 4 * T))


def kernel(x, gamma):
    x = np.asarray(x, dtype=np.float32)
    g = np.asarray(gamma, dtype=np.float32).reshape(-1)
    assert x.shape == (B, C, T), x.shape

    nc = _get_nc()
    idb = np.eye(128, dtype=ml_dtypes.bfloat16)
    diag = np.zeros((128, 2 * C), dtype=NP_FP8)
    idr = np.zeros((128, 2 * C), dtype=NP_FP8)
    # m-grouped: diag[j, m*C + kc*128 + j] = (m == kc)
    for m in range(2):
        for j in range(128):
            diag[j, m * C + m * 128 + j] = 1.0
    for m in range(2):
        for j in range(128):
            idr[j, m * C + m * 128 + j] = 1.0
    gb = np.full((128, 1), g[0], dtype=np.float32)
    in_maps = []
    for b in range(B):
        xtp, xr8p = _host_inputs(x[b], g)
        in_maps.append(
            {
                "xtp8": xtp,
                "xr8p": xr8p,
                "idb": idb,
                "id32": np.eye(128, dtype=np.float32),
                "diag8": diag,
                "identr": idr,
                "gamma_b": gb,
            }
        )

    trace = os.environ.get("KERNEL_TRACE", "0") == "1"
    res = run_bass_kernel_spmd(
        nc, in_maps, core_ids=list(range(N_CORES)), trace=trace
    )
    global LAST_RESULTS
    LAST_RESULTS = res
    return np.stack(
        [np.asarray(r["out"], dtype=np.float32) for r in res.results], axis=0
    )
